# revision 16
# baseline (speedup 1.0000x reference)
"""Trainium2 Bass kernel for LoopABMIL (attention-based MIL pooling).

reference:
    h = silu(x @ Wp + bp)            # [B, N, H]
    a = h @ Wa[:, 0] + ba            # [B, N]
    p = softmax(a masked to lengths) # [B, N]
    pooled = p @ h                   # [B, H]
    logits = pooled @ Wc + bc        # [B, C]

Softmax-pooling is associative: each of the 8 cores processes an arbitrary
subset of 128-patch chunks (round-robin over the global valid-chunk list, so
G = ceil(total_chunks/8) per core).  Each core emits an [8, 2*257] partial:
row b = two half-sums of (sum_p e^{a_p} h_p | sum_p e^{a_p}) over the core's
chunks of bag b.  The host sums the halves and the cores and applies the
tiny classifier.  ba cancels in the softmax ratio and is dropped on device.

Device structure:
  warm-up: HAM-warming matmuls sized to end when the first x superchunk
      lands.  The replicated-row constants (Wa broadcast, [bp|bp] bias
      matrix) are built on-device from a 1.5 KB DRAM row via ones-outer-
      product matmuls, keeping the early DMA rings free for Wp + x.  Four
      pair-bias matmuls are also pre-issued here into the rotating PSUM
      tiles.
  pass 1, per PAIR of chunks (one PSUM bank [128, 512]):
      1 bias matmul (skipped for the 4 pre-issued groups) + 16 accumulating
      k-tile matmuls + ONE Silu ACT into hstore (bf16, chunk stride 264;
      col 256 holds a memset 1.0 that yields the softmax denominator
      through the pooling matmul).  Per chunk, DVE mul + reduce gives the
      attention logit column a_g.
  exp is SPLIT so pooling can start the moment pass 1 ends (no PE idle,
  HAM stays at K=8/8): one batched mask-add + Exp over chunks [0, S1) is
  issued mid-pass (the ACT Silu<->Exp table switches hide in ACT slack),
  and the remainder after the last Silu.  39 pooling matmuls (lhsT = the
  [128, 8] w8 slice that is nonzero only in the chunk's bag column) then
  accumulate into two [8, 257] PSUM tiles; each half is copied and DMA'd
  out as soon as it completes.
"""

import sys

if "/opt/trn_rl_repo" not in sys.path:
    sys.path.insert(0, "/opt/trn_rl_repo")

from contextlib import ExitStack

import ml_dtypes
import numpy as np

import concourse.bacc as bacc
import concourse.tile as tile
from concourse import mybir
from concourse.bass_utils import run_bass_kernel_spmd

B, N, D, H, C = 8, 8192, 1024, 256, 2
P = 128          # patch chunk size (SBUF partitions)
NCORES = 8
KT = D // P      # k-tiles in the projection contraction
NEG = -30000.0   # additive mask: exp(a + NEG) == 0.0 exactly in f32
CHW = H + 8      # hstore per-chunk stride: 256 h cols + ones col + pad
SUP = 4          # chunks per DMA superchunk after the first
NWARM = 26       # HAM warm-up matmuls: end when the first superchunk lands
NPRE = 4         # pair-bias matmuls pre-issued during warm-up (= hprep bufs)

BF = mybir.dt.bfloat16
F32 = mybir.dt.float32

_cache: dict = {}


def _sched(G: int):
    sizes = [min(2, G)]
    while sum(sizes) < G:
        sizes.append(min(SUP, G - sum(sizes)))
    return sizes


def _build(G: int, act=None) -> "bacc.Bacc":
    """One SPMD program shared by all 8 cores: G chunk slots, bag routing
    entirely data-driven via mask8 (so the program depends only on G)."""
    if act is None:
        act = mybir.ActivationFunctionType.Silu
    S1 = min(32, G)          # chunks covered by the early exp
    nc = bacc.Bacc("TRN2", target_bir_lowering=False)

    xpk = nc.dram_tensor("xpk", [P, G * D], BF, kind="ExternalInput")
    cblob = nc.dram_tensor("cblob", [P, KT * H], BF, kind="ExternalInput")
    crow = nc.dram_tensor("crow", [1, 768], BF, kind="ExternalInput")
    mask8 = nc.dram_tensor("mask8", [P, G, 8], F32, kind="ExternalInput")
    out = nc.dram_tensor("out", [8, 2 * 257], F32, kind="ExternalOutput")

    with tile.TileContext(nc) as tc, ExitStack() as ctx:
        const = ctx.enter_context(tc.tile_pool(name="const", bufs=1))
        xpool = ctx.enter_context(tc.tile_pool(name="xp", bufs=6))
        hprep = ctx.enter_context(tc.tile_pool(name="hpre", bufs=NPRE, space="PSUM"))
        store = ctx.enter_context(tc.tile_pool(name="store", bufs=1))
        scrp = ctx.enter_context(tc.tile_pool(name="scr", bufs=2))
        warmp = ctx.enter_context(tc.tile_pool(name="warm", bufs=1, space="PSUM"))
        poolps = ctx.enter_context(tc.tile_pool(name="poolps", bufs=1, space="PSUM"))
        outp = ctx.enter_context(tc.tile_pool(name="outp", bufs=1))

        # crow first on the sync DMA queue: tiny, unblocks the on-device
        # constant builds below while Wp/x stream on the rings.
        crow_sb = const.tile([1, 768], BF, tag="crow")
        nc.sync.dma_start(out=crow_sb, in_=crow[:])

        # warm_in row 0 is all-ones: doubles as the bias-broadcast stationary
        # (full 128x128 LDWEIGHTS -> background weight buffer, no row-group
        # conflict stall at pair boundaries).
        warm_in = const.tile([P, P], BF, tag="warmin")
        nc.vector.memset(warm_in, 0.0)
        nc.vector.memset(warm_in[0:1, :], 1.0)
        wab_bc = const.tile([P, H], BF, tag="wab")
        bias_t = const.tile([P, 512], BF, tag="biast")

        # superchunk DMA: [2, 4, 4, ...], 6 tiles in flight; cblob (Wp) is
        # issued between sup0 and sup1.
        sizes = _sched(G)
        sup_start = {}
        g0 = 0
        for ns in sizes:
            sup_start[g0] = ns
            g0 += ns
        xsup = None
        sup_base = 0
        xts = [None] * G
        cb = None
        for g in range(G):
            if g in sup_start:
                ns = sup_start[g]
                sup_base = g
                xsup = xpool.tile([P, SUP * D], BF, tag="xt")
                nc.sync.dma_start(
                    out=xsup[:, 0:ns * D],
                    in_=xpk[:, g * D:(g + ns) * D],
                )
                if cb is None:
                    cb = const.tile([P, KT * H], BF, tag="cblob")
                    nc.sync.dma_start(out=cb, in_=cblob[:])
            xts[g] = xsup[:, (g - sup_base) * D:(g - sup_base + 1) * D]
        wp_t = [cb[:, k * H:(k + 1) * H] for k in range(KT)]
        mask_t = const.tile([P, G, 8], F32, tag="mask")
        nc.gpsimd.dma_start(out=mask_t, in_=mask8[:])

        hstore = store.tile([P, G, CHW], BF, tag="hstore")
        nc.vector.memset(hstore[:, :, H:H + 1], 1.0)  # denominator ones col
        a_all = store.tile([P, G], F32, tag="a_all")
        am8 = store.tile([P, G, 8], F32, tag="am8")
        w8 = store.tile([P, G, 8], BF, tag="w8")
        out_sb = outp.tile([8, 2 * 257], F32, tag="outsb")

        # ---- warm-up + on-device constant builds + pre-issued biases ----
        wps = warmp.tile([P, P], F32, tag="warmps")
        wb_ps = warmp.tile([P, 512], F32, tag="wbps")
        hp_pre = []

        def warm(i0, i1, n):
            for i in range(i0, i1):
                nc.tensor.matmul(
                    wps, lhsT=warm_in, rhs=warm_in,
                    start=(i == 0), stop=(i == n - 1),
                )

        warm(0, 8, NWARM)
        # Wa row -> broadcast [128, 256] via ones-outer-product
        nc.tensor.matmul(
            wb_ps[:, 0:H], lhsT=warm_in[0:1, :], rhs=crow_sb[0:1, 0:H],
            start=True, stop=True,
        )
        nc.vector.tensor_copy(wab_bc, wb_ps[:, 0:H])
        warm(8, 14, NWARM)
        # [bp|bp] rows -> bias matrix [128, 512] (row 0 real, rest zero
        # because warm_in rows 1..127 are zero)
        nc.tensor.matmul(
            wb_ps, lhsT=warm_in[0:1, :], rhs=crow_sb[0:1, H:H + 512],
            start=True, stop=True,
        )
        warm(14, 20, NWARM)
        nc.vector.tensor_copy(bias_t, wb_ps)
        warm(20, NWARM, NWARM)
        npre = min(NPRE, (G - 1) // 2)
        for j in range(npre):
            hp = hprep.tile([P, 2 * H], F32, tag="hp")
            nc.tensor.matmul(
                hp, lhsT=warm_in, rhs=bias_t,
                start=True, stop=False, skip_group_check=True,
            )
            hp_pre.append(hp)

        def attn(g):
            scr = scrp.tile([P, H], BF, tag="scrf")
            nc.vector.tensor_mul(scr, hstore[:, g, 0:H], wab_bc)
            nc.vector.reduce_sum(
                out=a_all[:, g:g + 1], in_=scr, axis=mybir.AxisListType.X
            )

        def exp_segment(lo, hi):
            nc.vector.tensor_add(
                am8[:, lo:hi],
                mask_t[:, lo:hi],
                a_all[:, lo:hi].broadcast_to((P, hi - lo, 8)),
            )
            nc.scalar.activation(
                out=w8[:, lo:hi], in_=am8[:, lo:hi],
                func=mybir.ActivationFunctionType.Exp,
            )

        # ---- pass 1: projection + silu + attention logits, paired chunks ----
        g = 0
        grp = 0
        while g < G:
            w = 2 if g + 1 < G else 1
            if grp < npre and w == 2:
                hp = hp_pre[grp]
            else:
                hp = hprep.tile([P, w * H], F32, tag="hp")
                nc.tensor.matmul(
                    hp, lhsT=warm_in, rhs=bias_t[:, 0:w * H],
                    start=True, stop=False, skip_group_check=True,
                )
            for c in range(w):
                xt = xts[g + c]
                for k in range(KT):
                    nc.tensor.matmul(
                        hp[:, c * H:(c + 1) * H],
                        lhsT=xt[:, k * P:(k + 1) * P],
                        rhs=wp_t[k],
                        start=False,
                        stop=(c == w - 1 and k == KT - 1),
                        skip_group_check=True,
                    )
            nc.scalar.activation(out=hstore[:, g:g + w, 0:H], in_=hp, func=act)
            for c in range(w):
                attn(g + c)
            g += w
            grp += 1
            if g == S1:
                exp_segment(0, S1)       # early exp: pools can start on time
        if S1 < G:
            exp_segment(S1, G)

        # ---- pooling: two accumulation tiles; copy+DMA each half early ----
        pool_a = poolps.tile([8, 257], F32, tag="poola")
        pool_b = poolps.tile([8, 257], F32, tag="poolb")
        for g in range(G):
            dst = pool_a if g < S1 else pool_b
            nc.tensor.matmul(
                dst,
                lhsT=w8[:, g, :],
                rhs=hstore[:, g, 0:257],
                start=(g == 0 or g == S1),
                stop=(g == S1 - 1 or g == G - 1),
                skip_group_check=True,
            )
            if g == S1 - 1:
                nc.vector.tensor_copy(out_sb[:, 0:257], pool_a)
                nc.sync.dma_start(out=out[:, 0:257], in_=out_sb[:, 0:257])
        if S1 < G:
            nc.scalar.copy(out_sb[:, 257:514], pool_b)
        else:
            nc.vector.memset(out_sb[:, 257:514], 0.0)
        nc.sync.dma_start(out=out[:, 257:514], in_=out_sb[:, 257:514])

    nc.compile()
    return nc


def _plan(lengths: np.ndarray):
    """Global valid-chunk list, round-robin across cores; G slots per core."""
    lens = np.asarray(lengths, dtype=np.int64)
    T = np.maximum((lens + P - 1) // P, 1)       # valid chunks per bag
    total = int(T.sum())
    G = (total + NCORES - 1) // NCORES
    bs = np.repeat(np.arange(B), T)              # bag id per global chunk
    ts = np.concatenate([np.arange(t) for t in T])  # chunk id within bag
    return T, G, bs, ts


def _pack(x, lengths, G, bs, ts):
    """Per-core inputs: xpk [128, G*1024] bf16 (lhsT layout) + mask8."""
    lens = np.asarray(lengths, dtype=np.int64)
    total = len(bs)
    # x[b, t*128+p, k*128+d] -> xr[b, t, d, k*128+p]  (d = within-k-tile idx)
    xr = (
        np.asarray(x)
        .astype(ml_dtypes.bfloat16)
        .reshape(B, N // P, P, KT, P)
        .transpose(0, 1, 4, 3, 2)
        .reshape(B, N // P, P, D)
    )
    in_maps = []
    for c in range(NCORES):
        idx = c + NCORES * np.arange(G)          # global chunk per slot
        real = idx < total
        idx_c = np.minimum(idx, total - 1)
        b_s, t_s = bs[idx_c], ts[idx_c]
        xpk = xr[b_s, t_s]                       # [G, 128, 1024] bf16
        xpk_t = np.ascontiguousarray(xpk.transpose(1, 0, 2)).reshape(P, G * D)
        valid = np.clip(lens[b_s] - t_s * P, 0, P)
        valid[~real] = 0
        # mask8[p, g, j]: 0 only for j == bag(g) and p < valid(g)
        m = np.full((P, G, 8), NEG, dtype=np.float32)
        ok = np.arange(P)[:, None] < valid[None, :]          # [P, G]
        m[:, np.arange(G), b_s] = np.where(ok, 0.0, NEG)
        in_maps.append({"xpk": xpk_t, "mask8": m})
    return in_maps


def _run(inputs: dict, trace: bool = False):
    x = np.asarray(inputs["x"], dtype=np.float32)
    lengths = np.asarray(inputs["lengths"])
    Wp = np.asarray(inputs["Wp"], dtype=np.float32)
    bp = np.asarray(inputs["bp"], dtype=np.float32)
    Wa = np.asarray(inputs["Wa"], dtype=np.float32)
    Wc = np.asarray(inputs["Wc"], dtype=np.float32)
    bc = np.asarray(inputs["bc"], dtype=np.float32)

    T, G, bs, ts = _plan(lengths)
    if G not in _cache:
        _cache[G] = _build(G)
    nc = _cache[G]

    in_maps = _pack(x, lengths, G, bs, ts)
    # cblob: wp k-tiles: cblob[d_in, k*H + h] = Wp[k*P + d_in, h]
    cblob = (
        Wp.reshape(KT, P, H).transpose(1, 0, 2).reshape(P, KT * H)
        .astype(ml_dtypes.bfloat16)
    )
    crow = np.concatenate([Wa[:, 0], bp, bp]).astype(ml_dtypes.bfloat16)
    crow = crow.reshape(1, 768)
    for m in in_maps:
        m["cblob"] = cblob
        m["crow"] = crow

    res = run_bass_kernel_spmd(
        nc, in_maps, core_ids=list(range(NCORES)), trace=trace
    )

    v = np.zeros((B, H), np.float64)
    s = np.zeros(B, np.float64)
    for r in res.results:
        o = r["out"].astype(np.float64)
        v += o[:, 0:H] + o[:, 257:257 + H]
        s += o[:, H] + o[:, 257 + H]
    pooled = v / s[:, None]
    logits = pooled @ Wc.astype(np.float64) + bc.astype(np.float64)
    return logits.astype(np.float32), res.exec_time_ns


def kernel(**inputs) -> np.ndarray:
    logits, _ = _run(inputs, trace=False)
    return logits


# revision 22
# speedup vs baseline: 1.0072x; 1.0072x over previous
"""Trainium2 Bass kernel for LoopABMIL (attention-based MIL pooling).

reference:
    h = silu(x @ Wp + bp)            # [B, N, H]
    a = h @ Wa[:, 0] + ba            # [B, N]
    p = softmax(a masked to lengths) # [B, N]
    pooled = p @ h                   # [B, H]
    logits = pooled @ Wc + bc        # [B, C]

Softmax-pooling is associative: each of the 8 cores processes an arbitrary
subset of 128-patch chunks (round-robin over the global valid-chunk list, so
G = ceil(total_chunks/8) per core).  Each core emits an [8, 2*257] partial:
row b = two half-sums of (sum_p e^{a_p} h_p | sum_p e^{a_p}) over the core's
chunks of bag b.  The host sums the halves and the cores and applies the
tiny classifier.  ba cancels in the softmax ratio and is dropped on device.

Device structure:
  warm-up: HAM-warming matmuls sized to end when the first x superchunk
      lands.  The replicated-row constants (Wa broadcast, [bp|bp] bias
      matrix) are built on-device from a 1.5 KB DRAM row via ones-outer-
      product matmuls, keeping the early DMA rings free for Wp + x.  Four
      pair-bias matmuls are also pre-issued here into the rotating PSUM
      tiles.
  pass 1, per PAIR of chunks (one PSUM bank [128, 512]):
      1 bias matmul (skipped for the 4 pre-issued groups) + 16 accumulating
      k-tile matmuls + ONE Silu ACT into hstore (bf16, chunk stride 264;
      col 256 holds a memset 1.0 that yields the softmax denominator
      through the pooling matmul).  Per chunk, DVE mul + reduce gives the
      attention logit column a_g.
  exp is SPLIT so pooling can start the moment pass 1 ends (no PE idle,
  HAM stays at K=8/8): one batched mask-add + Exp over chunks [0, S1) is
  issued mid-pass (the ACT Silu<->Exp table switches hide in ACT slack),
  and the remainder after the last Silu.  39 pooling matmuls (lhsT = the
  [128, 8] w8 slice that is nonzero only in the chunk's bag column) then
  accumulate into two [8, 257] PSUM tiles; each half is copied and DMA'd
  out as soon as it completes.
"""

import sys

if "/opt/trn_rl_repo" not in sys.path:
    sys.path.insert(0, "/opt/trn_rl_repo")

from contextlib import ExitStack

import ml_dtypes
import numpy as np

import concourse.bacc as bacc
import concourse.tile as tile
from concourse import mybir
from concourse.bass_utils import run_bass_kernel_spmd

B, N, D, H, C = 8, 8192, 1024, 256, 2
P = 128          # patch chunk size (SBUF partitions)
NCORES = 8
KT = D // P      # k-tiles in the projection contraction
NEG = -30000.0   # additive mask: exp(a + NEG) == 0.0 exactly in f32
CHW = H + 8      # hstore per-chunk stride: 256 h cols + ones col + pad
SUP = 4          # chunks per DMA superchunk after the first
NWARM = 33       # HAM warm-up matmuls: end when the first superchunk lands
NPRE = 4         # pair-bias matmuls pre-issued during warm-up (= hprep bufs)

BF = mybir.dt.bfloat16
F32 = mybir.dt.float32

_cache: dict = {}


def _sched(G: int):
    sizes = []
    for w in (2, 2, 2):
        if sum(sizes) < G:
            sizes.append(min(w, G - sum(sizes)))
    while sum(sizes) < G:
        sizes.append(min(SUP, G - sum(sizes)))
    return sizes


def _build(G: int, act=None) -> "bacc.Bacc":
    """One SPMD program shared by all 8 cores: G chunk slots, bag routing
    entirely data-driven via mask8 (so the program depends only on G)."""
    if act is None:
        act = mybir.ActivationFunctionType.Silu
    S1 = min(32, G)          # chunks covered by the early exp
    nc = bacc.Bacc("TRN2", target_bir_lowering=False)

    xpk = nc.dram_tensor("xpk", [P, G * D], BF, kind="ExternalInput")
    cblob = nc.dram_tensor("cblob", [P, KT * H + H], BF, kind="ExternalInput")
    crow = nc.dram_tensor("crow", [1, 512], BF, kind="ExternalInput")
    mask8 = nc.dram_tensor("mask8", [P, G, 8], F32, kind="ExternalInput")
    out = nc.dram_tensor("out", [8, 2 * 257], F32, kind="ExternalOutput")

    with tile.TileContext(nc) as tc, ExitStack() as ctx:
        const = ctx.enter_context(tc.tile_pool(name="const", bufs=1))
        xpool = ctx.enter_context(tc.tile_pool(name="xp", bufs=6))
        hprep = ctx.enter_context(tc.tile_pool(name="hpre", bufs=NPRE, space="PSUM"))
        store = ctx.enter_context(tc.tile_pool(name="store", bufs=1))
        scrp = ctx.enter_context(tc.tile_pool(name="scr", bufs=2))
        warmp = ctx.enter_context(tc.tile_pool(name="warm", bufs=1, space="PSUM"))
        poolps = ctx.enter_context(tc.tile_pool(name="poolps", bufs=1, space="PSUM"))
        outp = ctx.enter_context(tc.tile_pool(name="outp", bufs=1))

        # bias matrix [128, 512]: row 0 = [bp|bp] via a tiny DMA, rows 1..127
        # zeroed on-device -- keeps 131 KB of zeros off the early DMA rings.
        bias_t = const.tile([P, 512], BF, tag="biast")
        nc.vector.memset(bias_t, 0.0)
        nc.sync.dma_start(out=bias_t[0:1, :], in_=crow[:])

        # warm_in row 0 is all-ones: doubles as the bias-broadcast stationary
        # (full 128x128 LDWEIGHTS -> background weight buffer, no row-group
        # conflict stall at pair boundaries).
        warm_in = const.tile([P, P], BF, tag="warmin")
        nc.vector.memset(warm_in, 0.0)
        nc.vector.memset(warm_in[0:1, :], 1.0)

        # ring order: crow (above), then Wp|Wa blob, then x superchunks
        cb = const.tile([P, KT * H + H], BF, tag="cblob")
        nc.sync.dma_start(out=cb, in_=cblob[:])
        wp_t = [cb[:, k * H:(k + 1) * H] for k in range(KT)]
        wab_bc = cb[:, KT * H:KT * H + H]

        sizes = _sched(G)
        sup_start = {}
        g0 = 0
        for ns in sizes:
            sup_start[g0] = ns
            g0 += ns
        xsup = None
        sup_base = 0
        xts = [None] * G
        for g in range(G):
            if g in sup_start:
                ns = sup_start[g]
                sup_base = g
                xsup = xpool.tile([P, SUP * D], BF, tag="xt")
                nc.sync.dma_start(
                    out=xsup[:, 0:ns * D],
                    in_=xpk[:, g * D:(g + ns) * D],
                )
            xts[g] = xsup[:, (g - sup_base) * D:(g - sup_base + 1) * D]
        mask_t = const.tile([P, G, 8], F32, tag="mask")
        nc.gpsimd.dma_start(out=mask_t, in_=mask8[:])

        hstore = store.tile([P, G, CHW], BF, tag="hstore")
        nc.vector.memset(hstore[:, :, H:H + 1], 1.0)  # denominator ones col
        a_all = store.tile([P, G], F32, tag="a_all")
        am8 = store.tile([P, G, 8], F32, tag="am8")
        w8 = store.tile([P, G, 8], BF, tag="w8")
        out_sb = outp.tile([8, 2 * 257], F32, tag="outsb")

        # ---- warm-up (unbroken chain -> HAM K=8/8) + pre-issued biases ----
        wps = warmp.tile([P, P], F32, tag="warmps")
        hp_pre = []
        for i in range(NWARM):
            nc.tensor.matmul(
                wps, lhsT=warm_in, rhs=warm_in,
                start=(i == 0), stop=(i == NWARM - 1),
            )
        npre = min(NPRE, (G - 1) // 2)
        for j in range(npre):
            hp = hprep.tile([P, 2 * H], F32, tag="hp")
            nc.tensor.matmul(
                hp, lhsT=warm_in, rhs=bias_t,
                start=True, stop=False, skip_group_check=True,
            )
            hp_pre.append(hp)

        def attn(g):
            scr = scrp.tile([P, H], BF, tag="scrf")
            nc.vector.tensor_mul(scr, hstore[:, g, 0:H], wab_bc)
            nc.vector.reduce_sum(
                out=a_all[:, g:g + 1], in_=scr, axis=mybir.AxisListType.X
            )

        def exp_segment(lo, hi):
            nc.vector.tensor_add(
                am8[:, lo:hi],
                mask_t[:, lo:hi],
                a_all[:, lo:hi].broadcast_to((P, hi - lo, 8)),
            )
            nc.scalar.activation(
                out=w8[:, lo:hi], in_=am8[:, lo:hi],
                func=mybir.ActivationFunctionType.Exp,
            )

        # ---- pass 1: projection + silu + attention logits, paired chunks ----
        g = 0
        grp = 0
        while g < G:
            w = 2 if g + 1 < G else 1
            if grp < npre and w == 2:
                hp = hp_pre[grp]
            else:
                hp = hprep.tile([P, w * H], F32, tag="hp")
                nc.tensor.matmul(
                    hp, lhsT=warm_in, rhs=bias_t[:, 0:w * H],
                    start=True, stop=False, skip_group_check=True,
                )
            for c in range(w):
                xt = xts[g + c]
                for k in range(KT):
                    nc.tensor.matmul(
                        hp[:, c * H:(c + 1) * H],
                        lhsT=xt[:, k * P:(k + 1) * P],
                        rhs=wp_t[k],
                        start=False,
                        stop=(c == w - 1 and k == KT - 1),
                        skip_group_check=True,
                    )
            nc.scalar.activation(out=hstore[:, g:g + w, 0:H], in_=hp, func=act)
            for c in range(w):
                attn(g + c)
            g += w
            grp += 1
            if g == S1:
                exp_segment(0, S1)       # early exp: pools can start on time
        if S1 < G:
            exp_segment(S1, G)

        # ---- pooling: two accumulation tiles; copy+DMA each half early ----
        pool_a = poolps.tile([8, 257], F32, tag="poola")
        pool_b = poolps.tile([8, 257], F32, tag="poolb")
        for g in range(G):
            dst = pool_a if g < S1 else pool_b
            nc.tensor.matmul(
                dst,
                lhsT=w8[:, g, :],
                rhs=hstore[:, g, 0:257],
                start=(g == 0 or g == S1),
                stop=(g == S1 - 1 or g == G - 1),
                skip_group_check=True,
            )
            if g == S1 - 1:
                nc.vector.tensor_copy(out_sb[:, 0:257], pool_a)
                nc.sync.dma_start(out=out[:, 0:257], in_=out_sb[:, 0:257])
        if S1 < G:
            nc.scalar.copy(out_sb[:, 257:514], pool_b)
        else:
            nc.vector.memset(out_sb[:, 257:514], 0.0)
        nc.sync.dma_start(out=out[:, 257:514], in_=out_sb[:, 257:514])

    nc.compile()
    return nc


def _plan(lengths: np.ndarray):
    """Global valid-chunk list, round-robin across cores; G slots per core."""
    lens = np.asarray(lengths, dtype=np.int64)
    T = np.maximum((lens + P - 1) // P, 1)       # valid chunks per bag
    total = int(T.sum())
    G = (total + NCORES - 1) // NCORES
    bs = np.repeat(np.arange(B), T)              # bag id per global chunk
    ts = np.concatenate([np.arange(t) for t in T])  # chunk id within bag
    return T, G, bs, ts


def _pack(x, lengths, G, bs, ts):
    """Per-core inputs: xpk [128, G*1024] bf16 (lhsT layout) + mask8."""
    lens = np.asarray(lengths, dtype=np.int64)
    total = len(bs)
    # x[b, t*128+p, k*128+d] -> xr[b, t, d, k*128+p]  (d = within-k-tile idx)
    xr = (
        np.asarray(x)
        .astype(ml_dtypes.bfloat16)
        .reshape(B, N // P, P, KT, P)
        .transpose(0, 1, 4, 3, 2)
        .reshape(B, N // P, P, D)
    )
    in_maps = []
    for c in range(NCORES):
        idx = c + NCORES * np.arange(G)          # global chunk per slot
        real = idx < total
        idx_c = np.minimum(idx, total - 1)
        b_s, t_s = bs[idx_c], ts[idx_c]
        xpk = xr[b_s, t_s]                       # [G, 128, 1024] bf16
        xpk_t = np.ascontiguousarray(xpk.transpose(1, 0, 2)).reshape(P, G * D)
        valid = np.clip(lens[b_s] - t_s * P, 0, P)
        valid[~real] = 0
        # mask8[p, g, j]: 0 only for j == bag(g) and p < valid(g)
        m = np.full((P, G, 8), NEG, dtype=np.float32)
        ok = np.arange(P)[:, None] < valid[None, :]          # [P, G]
        m[:, np.arange(G), b_s] = np.where(ok, 0.0, NEG)
        in_maps.append({"xpk": xpk_t, "mask8": m})
    return in_maps


def _run(inputs: dict, trace: bool = False):
    x = np.asarray(inputs["x"], dtype=np.float32)
    lengths = np.asarray(inputs["lengths"])
    Wp = np.asarray(inputs["Wp"], dtype=np.float32)
    bp = np.asarray(inputs["bp"], dtype=np.float32)
    Wa = np.asarray(inputs["Wa"], dtype=np.float32)
    Wc = np.asarray(inputs["Wc"], dtype=np.float32)
    bc = np.asarray(inputs["bc"], dtype=np.float32)

    T, G, bs, ts = _plan(lengths)
    if G not in _cache:
        _cache[G] = _build(G)
    nc = _cache[G]

    in_maps = _pack(x, lengths, G, bs, ts)
    # cblob: wp k-tiles (cblob[d_in, k*H + h] = Wp[k*P + d_in, h]) | Wa bcast
    cblob = np.empty((P, KT * H + H), dtype=ml_dtypes.bfloat16)
    cblob[:, 0:KT * H] = (
        Wp.reshape(KT, P, H).transpose(1, 0, 2).reshape(P, KT * H)
        .astype(ml_dtypes.bfloat16)
    )
    cblob[:, KT * H:] = np.tile(
        Wa[:, 0][None, :], (P, 1)
    ).astype(ml_dtypes.bfloat16)
    crow = np.tile(bp, 2).astype(ml_dtypes.bfloat16).reshape(1, 512)
    for m in in_maps:
        m["cblob"] = cblob
        m["crow"] = crow

    res = run_bass_kernel_spmd(
        nc, in_maps, core_ids=list(range(NCORES)), trace=trace
    )

    v = np.zeros((B, H), np.float64)
    s = np.zeros(B, np.float64)
    for r in res.results:
        o = r["out"].astype(np.float64)
        v += o[:, 0:H] + o[:, 257:257 + H]
        s += o[:, H] + o[:, 257 + H]
    pooled = v / s[:, None]
    logits = pooled @ Wc.astype(np.float64) + bc.astype(np.float64)
    return logits.astype(np.float32), res.exec_time_ns


def kernel(**inputs) -> np.ndarray:
    logits, _ = _run(inputs, trace=False)
    return logits


# revision 28
# speedup vs baseline: 1.0086x; 1.0014x over previous
"""Trainium2 Bass kernel for LoopABMIL (attention-based MIL pooling).

reference:
    h = silu(x @ Wp + bp)            # [B, N, H]
    a = h @ Wa[:, 0] + ba            # [B, N]
    p = softmax(a masked to lengths) # [B, N]
    pooled = p @ h                   # [B, H]
    logits = pooled @ Wc + bc        # [B, C]

Softmax-pooling is associative: each of the 8 cores processes an arbitrary
subset of 128-patch chunks (round-robin over the global valid-chunk list, so
G = ceil(total_chunks/8) per core).  Each core emits an [8, 2*257] partial:
row b = two half-sums of (sum_p e^{a_p} h_p | sum_p e^{a_p}) over the core's
chunks of bag b.  The host sums the halves and the cores and applies the
tiny classifier.  ba cancels in the softmax ratio and is dropped on device.

Device structure:
  warm-up: HAM-warming matmuls sized to end when the first x superchunk
      lands.  The replicated-row constants (Wa broadcast, [bp|bp] bias
      matrix) are built on-device from a 1.5 KB DRAM row via ones-outer-
      product matmuls, keeping the early DMA rings free for Wp + x.  Four
      pair-bias matmuls are also pre-issued here into the rotating PSUM
      tiles.
  pass 1, per PAIR of chunks (one PSUM bank [128, 512]):
      1 bias matmul (skipped for the 4 pre-issued groups) + 16 accumulating
      k-tile matmuls + ONE Silu ACT into hstore (bf16, chunk stride 264;
      col 256 holds a memset 1.0 that yields the softmax denominator
      through the pooling matmul).  Per chunk, DVE mul + reduce gives the
      attention logit column a_g.
  exp is SPLIT so pooling can start the moment pass 1 ends (no PE idle,
  HAM stays at K=8/8): one batched mask-add + Exp over chunks [0, S1) is
  issued mid-pass (the ACT Silu<->Exp table switches hide in ACT slack),
  and the remainder after the last Silu.  39 pooling matmuls (lhsT = the
  [128, 8] w8 slice that is nonzero only in the chunk's bag column) then
  accumulate into two [8, 257] PSUM tiles; each half is copied and DMA'd
  out as soon as it completes.
"""

import sys

if "/opt/trn_rl_repo" not in sys.path:
    sys.path.insert(0, "/opt/trn_rl_repo")

from contextlib import ExitStack

import ml_dtypes
import numpy as np

import concourse.bacc as bacc
import concourse.tile as tile
from concourse import mybir
from concourse.bass_utils import run_bass_kernel_spmd

B, N, D, H, C = 8, 8192, 1024, 256, 2
P = 128          # patch chunk size (SBUF partitions)
NCORES = 8
KT = D // P      # k-tiles in the projection contraction
NEG = -30000.0   # additive mask: exp(a + NEG) == 0.0 exactly in f32
CHW = H + 8      # hstore per-chunk stride: 256 h cols + ones col + pad
SUP = 4          # chunks per DMA superchunk after the first
NWARM = 30       # HAM warm-up matmuls: end when the first superchunk lands

BF = mybir.dt.bfloat16
F32 = mybir.dt.float32

_cache: dict = {}


def _sched(G: int):
    sizes = []
    for w in (2, 2, 2):
        if sum(sizes) < G:
            sizes.append(min(w, G - sum(sizes)))
    while sum(sizes) < G:
        sizes.append(min(SUP, G - sum(sizes)))
    return sizes


def _build(G: int, act=None) -> "bacc.Bacc":
    """One SPMD program shared by all 8 cores: G chunk slots, bag routing
    entirely data-driven via mask8 (so the program depends only on G)."""
    if act is None:
        act = mybir.ActivationFunctionType.Silu
    S1 = min(32, G)          # chunks covered by the early exp
    nc = bacc.Bacc("TRN2", target_bir_lowering=False)

    xpk = nc.dram_tensor("xpk", [P, G * D], BF, kind="ExternalInput")
    cblob = nc.dram_tensor("cblob", [P, KT * H + H], BF, kind="ExternalInput")
    crow = nc.dram_tensor("crow", [1, 512], BF, kind="ExternalInput")
    mask8 = nc.dram_tensor("mask8", [P, G, 8], F32, kind="ExternalInput")
    out = nc.dram_tensor("out", [8, 2 * 257], F32, kind="ExternalOutput")

    with tile.TileContext(nc) as tc, ExitStack() as ctx:
        const = ctx.enter_context(tc.tile_pool(name="const", bufs=1))
        xpool = ctx.enter_context(tc.tile_pool(name="xp", bufs=6))
        hprep = ctx.enter_context(tc.tile_pool(name="hpre", bufs=4, space="PSUM"))
        store = ctx.enter_context(tc.tile_pool(name="store", bufs=1))
        scrp = ctx.enter_context(tc.tile_pool(name="scr", bufs=2))
        warmp = ctx.enter_context(tc.tile_pool(name="warm", bufs=1, space="PSUM"))
        poolps = ctx.enter_context(tc.tile_pool(name="poolps", bufs=1, space="PSUM"))
        outp = ctx.enter_context(tc.tile_pool(name="outp", bufs=1))

        # bias matrix [128, 512]: row 0 = [bp|bp] via a tiny DMA, rows 1..127
        # zeroed on-device -- keeps 131 KB of zeros off the early DMA rings.
        bias_t = const.tile([P, 512], BF, tag="biast")
        nc.vector.memset(bias_t, 0.0)
        nc.sync.dma_start(out=bias_t[0:1, :], in_=crow[:])

        # warm_in row 0 is all-ones: doubles as the bias-broadcast stationary
        # (full 128x128 LDWEIGHTS -> background weight buffer, no row-group
        # conflict stall at pair boundaries).
        warm_in = const.tile([P, P], BF, tag="warmin")
        nc.vector.memset(warm_in, 0.0)
        nc.vector.memset(warm_in[0:1, :], 1.0)

        # ring order: crow (above), then Wp|Wa blob, then x superchunks
        cb = const.tile([P, KT * H + H], BF, tag="cblob")
        nc.sync.dma_start(out=cb, in_=cblob[:])
        wp_t = [cb[:, k * H:(k + 1) * H] for k in range(KT)]
        wab_bc = cb[:, KT * H:KT * H + H]

        sizes = _sched(G)
        sup_start = {}
        g0 = 0
        for ns in sizes:
            sup_start[g0] = ns
            g0 += ns
        xsup = None
        sup_base = 0
        xts = [None] * G
        for g in range(G):
            if g in sup_start:
                ns = sup_start[g]
                sup_base = g
                xsup = xpool.tile([P, SUP * D], BF, tag="xt")
                # first superchunk rides the software DGE queue: its early
                # bandwidth is additive with the hardware rings carrying Wp
                eng = nc.gpsimd if g == 0 else nc.sync
                eng.dma_start(
                    out=xsup[:, 0:ns * D],
                    in_=xpk[:, g * D:(g + ns) * D],
                )
            xts[g] = xsup[:, (g - sup_base) * D:(g - sup_base + 1) * D]
        mask_t = const.tile([P, G, 8], F32, tag="mask")
        nc.gpsimd.dma_start(out=mask_t, in_=mask8[:])

        hstore = store.tile([P, G, CHW], BF, tag="hstore")
        nc.vector.memset(hstore[:, :, H:H + 1], 1.0)  # denominator ones col
        a_all = store.tile([P, G], F32, tag="a_all")
        am8 = store.tile([P, G, 8], F32, tag="am8")
        w8 = store.tile([P, G, 8], BF, tag="w8")
        out_sb = outp.tile([8, 2 * 257], F32, tag="outsb")

        # ---- warm-up (unbroken chain -> HAM K=8/8) ----
        wps = warmp.tile([P, P], F32, tag="warmps")
        for i in range(NWARM):
            nc.tensor.matmul(
                wps, lhsT=warm_in, rhs=warm_in,
                start=(i == 0), stop=(i == NWARM - 1),
            )

        def attn(g):
            scr = scrp.tile([P, H], BF, tag="scrf")
            nc.vector.tensor_mul(scr, hstore[:, g, 0:H], wab_bc)
            nc.vector.reduce_sum(
                out=a_all[:, g:g + 1], in_=scr, axis=mybir.AxisListType.X
            )

        def exp_segment(lo, hi):
            nc.vector.tensor_add(
                am8[:, lo:hi],
                mask_t[:, lo:hi],
                a_all[:, lo:hi].broadcast_to((P, hi - lo, 8)),
            )
            nc.scalar.activation(
                out=w8[:, lo:hi], in_=am8[:, lo:hi],
                func=mybir.ActivationFunctionType.Exp,
            )

        # ---- pass 1: projection + silu + attention logits, paired chunks ----
        g = 0
        while g < G:
            w = 2 if g + 1 < G else 1
            hp = hprep.tile([P, w * H], F32, tag="hp")
            nc.tensor.matmul(
                hp, lhsT=warm_in, rhs=bias_t[:, 0:w * H],
                start=True, stop=False, skip_group_check=True,
            )
            for c in range(w):
                xt = xts[g + c]
                for k in range(KT):
                    nc.tensor.matmul(
                        hp[:, c * H:(c + 1) * H],
                        lhsT=xt[:, k * P:(k + 1) * P],
                        rhs=wp_t[k],
                        start=False,
                        stop=(c == w - 1 and k == KT - 1),
                        skip_group_check=True,
                    )
            nc.scalar.activation(out=hstore[:, g:g + w, 0:H], in_=hp, func=act)
            for c in range(w):
                attn(g + c)
            g += w
            if g == S1:
                exp_segment(0, S1)       # early exp: pools can start on time
        if S1 < G:
            exp_segment(S1, G)

        # ---- pooling: two accumulation tiles; copy+DMA each half early ----
        pool_a = poolps.tile([8, 257], F32, tag="poola")
        pool_b = poolps.tile([8, 257], F32, tag="poolb")
        for g in range(G):
            dst = pool_a if g < S1 else pool_b
            nc.tensor.matmul(
                dst,
                lhsT=w8[:, g, :],
                rhs=hstore[:, g, 0:257],
                start=(g == 0 or g == S1),
                stop=(g == S1 - 1 or g == G - 1),
                skip_group_check=True,
            )
            if g == S1 - 1:
                nc.vector.tensor_copy(out_sb[:, 0:257], pool_a)
                nc.sync.dma_start(out=out[:, 0:257], in_=out_sb[:, 0:257])
        if S1 < G:
            nc.scalar.copy(out_sb[:, 257:514], pool_b)
        else:
            nc.vector.memset(out_sb[:, 257:514], 0.0)
        nc.sync.dma_start(out=out[:, 257:514], in_=out_sb[:, 257:514])

    nc.compile()
    return nc


def _plan(lengths: np.ndarray):
    """Global valid-chunk list, round-robin across cores; G slots per core."""
    lens = np.asarray(lengths, dtype=np.int64)
    T = np.maximum((lens + P - 1) // P, 1)       # valid chunks per bag
    total = int(T.sum())
    G = (total + NCORES - 1) // NCORES
    bs = np.repeat(np.arange(B), T)              # bag id per global chunk
    ts = np.concatenate([np.arange(t) for t in T])  # chunk id within bag
    return T, G, bs, ts


def _pack(x, lengths, G, bs, ts):
    """Per-core inputs: xpk [128, G*1024] bf16 (lhsT layout) + mask8."""
    lens = np.asarray(lengths, dtype=np.int64)
    total = len(bs)
    # x[b, t*128+p, k*128+d] -> xr[b, t, d, k*128+p]  (d = within-k-tile idx)
    xr = (
        np.asarray(x)
        .astype(ml_dtypes.bfloat16)
        .reshape(B, N // P, P, KT, P)
        .transpose(0, 1, 4, 3, 2)
        .reshape(B, N // P, P, D)
    )
    in_maps = []
    for c in range(NCORES):
        idx = c + NCORES * np.arange(G)          # global chunk per slot
        real = idx < total
        idx_c = np.minimum(idx, total - 1)
        b_s, t_s = bs[idx_c], ts[idx_c]
        xpk = xr[b_s, t_s]                       # [G, 128, 1024] bf16
        xpk_t = np.ascontiguousarray(xpk.transpose(1, 0, 2)).reshape(P, G * D)
        valid = np.clip(lens[b_s] - t_s * P, 0, P)
        valid[~real] = 0
        # mask8[p, g, j]: 0 only for j == bag(g) and p < valid(g)
        m = np.full((P, G, 8), NEG, dtype=np.float32)
        ok = np.arange(P)[:, None] < valid[None, :]          # [P, G]
        m[:, np.arange(G), b_s] = np.where(ok, 0.0, NEG)
        in_maps.append({"xpk": xpk_t, "mask8": m})
    return in_maps


def _run(inputs: dict, trace: bool = False):
    x = np.asarray(inputs["x"], dtype=np.float32)
    lengths = np.asarray(inputs["lengths"])
    Wp = np.asarray(inputs["Wp"], dtype=np.float32)
    bp = np.asarray(inputs["bp"], dtype=np.float32)
    Wa = np.asarray(inputs["Wa"], dtype=np.float32)
    Wc = np.asarray(inputs["Wc"], dtype=np.float32)
    bc = np.asarray(inputs["bc"], dtype=np.float32)

    T, G, bs, ts = _plan(lengths)
    if G not in _cache:
        _cache[G] = _build(G)
    nc = _cache[G]

    in_maps = _pack(x, lengths, G, bs, ts)
    # cblob: wp k-tiles (cblob[d_in, k*H + h] = Wp[k*P + d_in, h]) | Wa bcast
    cblob = np.empty((P, KT * H + H), dtype=ml_dtypes.bfloat16)
    cblob[:, 0:KT * H] = (
        Wp.reshape(KT, P, H).transpose(1, 0, 2).reshape(P, KT * H)
        .astype(ml_dtypes.bfloat16)
    )
    cblob[:, KT * H:] = np.tile(
        Wa[:, 0][None, :], (P, 1)
    ).astype(ml_dtypes.bfloat16)
    crow = np.tile(bp, 2).astype(ml_dtypes.bfloat16).reshape(1, 512)
    for m in in_maps:
        m["cblob"] = cblob
        m["crow"] = crow

    res = run_bass_kernel_spmd(
        nc, in_maps, core_ids=list(range(NCORES)), trace=trace
    )

    v = np.zeros((B, H), np.float64)
    s = np.zeros(B, np.float64)
    for r in res.results:
        o = r["out"].astype(np.float64)
        v += o[:, 0:H] + o[:, 257:257 + H]
        s += o[:, H] + o[:, 257 + H]
    pooled = v / s[:, None]
    logits = pooled @ Wc.astype(np.float64) + bc.astype(np.float64)
    return logits.astype(np.float32), res.exec_time_ns


def kernel(**inputs) -> np.ndarray:
    logits, _ = _run(inputs, trace=False)
    return logits
